# revision 3
# baseline (speedup 1.0000x reference)
"""GCN layer (SpMM) Trainium2 kernel: out = segment_sum(vals * x[cols], rows).

Self-contained: host-side sharding/preprocessing + a uniform Bass/Tile
program run SPMD on 8 NeuronCores via bass_utils.run_bass_kernel_spmd.

Strategy (graph/data-parallel SpMM per the row-partition scheme):
  - adj_rows is sorted; each core takes a contiguous 1/8 row range (its
    edges are then contiguous).
  - Neighbor features are fetched with the SWDGE `dma_gather` (256B rows
    from HBM by index). Its indices are int16, so x is split into CH=4
    chunks of 25K rows; each edge is routed to its column's chunk.
  - Per core, whole rows are packed into groups (<=64 distinct rows, and
    <=256 edges per chunk); each (group, chunk) segment is padded to
    exactly 256 tokens. rel[token] = dense rank of its row in the group.
  - Gather calls batch SG=8 groups per chunk (2048 indices/call).
  - Reduction per group on-chip: for each 128-token tile, DVE builds a
    one-hot S[p,w] = (iota[w] == rel[p]) and scales the gathered rows by
    val; PE accumulates S^T @ (tok*val) into a PSUM [64,64] accumulator;
    the result is staged in SBUF and written out in batches.
  - Host scatters staged group rows back to out[row] (rows are unique
    across groups/cores, so no collisions).
"""
import numpy as np

D = 64
P = 128
N_CORES = 8
CH = 4
TPC = 2
SG = 8
CAP_ROWS = 64


# ---------------------------------------------------------------- host side
def _pack_core(rows, cols, vals, r_lo, r_hi, G, chunk_rows):
    seg_cap = TPC * P
    e_lo = np.searchsorted(rows, r_lo, "left")
    e_hi = np.searchsorted(rows, r_hi, "left")
    r = rows[e_lo:e_hi].astype(np.int64)
    c = cols[e_lo:e_hi].astype(np.int64)
    v = vals[e_lo:e_hi].astype(np.float32)
    ch = c // chunk_rows
    n_rows_core = r_hi - r_lo
    rr = r - r_lo
    cum = np.zeros((CH, n_rows_core + 1), np.int64)
    for cc in range(CH):
        cum[cc, 1:] = np.cumsum(np.bincount(rr[ch == cc], minlength=n_rows_core))
    starts = []
    s = 0
    while s < n_rows_core:
        hi = min(s + CAP_ROWS, n_rows_core)
        k = hi - s
        for cc in range(CH):
            kk = np.searchsorted(cum[cc, s + 1 : hi + 1] - cum[cc, s],
                                 seg_cap, "right")
            k = min(k, kk)
        if k == 0:
            raise ValueError("row degree exceeds segment capacity")
        starts.append((s, k))
        s += k
    assert len(starts) <= G, (len(starts), G)

    order = np.argsort(ch, kind="stable")
    r_s, c_s, v_s, ch_s = rr[order], c[order], v[order], ch[order]
    chunk_lo = np.searchsorted(ch_s, np.arange(CH), "left")
    chunk_hi = np.searchsorted(ch_s, np.arange(CH), "right")

    idx_lin = np.zeros((G, CH, seg_cap), np.int16)
    rel_lin = np.zeros((G, CH, seg_cap), np.float32)
    val_lin = np.zeros((G, CH, seg_cap), np.float32)
    row_of = np.full((G, CAP_ROWS), -1, np.int64)

    rank = np.zeros(n_rows_core, np.int64)
    gid = np.zeros(n_rows_core, np.int64)
    for g, (s, k) in enumerate(starts):
        rank[s : s + k] = np.arange(k)
        gid[s : s + k] = g
        row_of[g, :k] = r_lo + s + np.arange(k)

    n_groups_real = len(starts)
    for cc in range(CH):
        lo, hi = chunk_lo[cc], chunk_hi[cc]
        rcc = r_s[lo:hi]
        icc = (c_s[lo:hi] - cc * chunk_rows).astype(np.int16)
        vcc = v_s[lo:hi]
        relcc = rank[rcc].astype(np.float32)
        gcc = gid[rcc]
        grp_start = np.searchsorted(gcc, np.arange(n_groups_real), "left")
        pos = np.arange(hi - lo) - grp_start[gcc]
        idx_lin[gcc, cc, pos] = icc
        rel_lin[gcc, cc, pos] = relcc
        val_lin[gcc, cc, pos] = vcc

    ncol = G * CH * TPC
    rel_all = np.zeros((P, ncol), np.float32)
    val_all = np.zeros((P, ncol), np.float32)
    k_col = (
        np.arange(G)[:, None, None] * (CH * TPC)
        + np.arange(CH)[None, :, None] * TPC
        + (np.arange(seg_cap)[None, None, :] // P)
    )
    p_col = np.arange(seg_cap)[None, None, :] % P
    rel_all[p_col, k_col] = rel_lin
    val_all[p_col, k_col] = val_lin

    n_call_tok = SG * seg_cap
    n_sg = G // SG
    ccols = n_call_tok // 16
    idx_all = np.zeros((P, n_sg * CH * ccols), np.int16)
    for sg in range(n_sg):
        for cc in range(CH):
            lin = idx_lin[sg * SG : (sg + 1) * SG, cc, :].reshape(-1)
            blk = lin.reshape(ccols, 16).T
            col0 = (sg * CH + cc) * ccols
            idx_all[:, col0 : col0 + ccols] = np.tile(blk, (P // 16, 1))

    iota = np.broadcast_to(np.arange(CAP_ROWS, dtype=np.float32), (P, CAP_ROWS))
    meta = np.ascontiguousarray(np.concatenate([iota, rel_all, val_all], 1))
    return idx_all, meta, row_of


def _count_groups(rows, cols, r_lo, r_hi, chunk_rows):
    seg_cap = TPC * P
    e_lo = np.searchsorted(rows, r_lo, "left")
    e_hi = np.searchsorted(rows, r_hi, "left")
    r = rows[e_lo:e_hi].astype(np.int64) - r_lo
    c = cols[e_lo:e_hi].astype(np.int64)
    ch = c // chunk_rows
    n_rows_core = r_hi - r_lo
    cum = np.zeros((CH, n_rows_core + 1), np.int64)
    for cc in range(CH):
        cum[cc, 1:] = np.cumsum(np.bincount(r[ch == cc], minlength=n_rows_core))
    s, n = 0, 0
    while s < n_rows_core:
        hi = min(s + CAP_ROWS, n_rows_core)
        k = hi - s
        for cc in range(CH):
            kk = np.searchsorted(cum[cc, s + 1 : hi + 1] - cum[cc, s],
                                 seg_cap, "right")
            k = min(k, kk)
        if k == 0:
            raise ValueError("row degree exceeds segment capacity")
        s += k
        n += 1
    return n


# ---------------------------------------------------------------- device side
def _build_program(n_x_rows_padded, G, chunk_rows, tok_bufs=2, work_bufs=4,
                   psum_bufs=4):
    import concourse.bacc as bacc
    import concourse.mybir as mybir
    import concourse.tile as tile

    n_sg = G // SG
    seg_cap = TPC * P
    n_call_tok = SG * seg_cap
    ccols = n_call_tok // 16
    ncol = G * CH * TPC

    nc = bacc.Bacc(None)
    x_t = nc.dram_tensor("x", [n_x_rows_padded, D], mybir.dt.float32,
                         kind="ExternalInput")
    idx_t = nc.dram_tensor("idx", [P, n_sg * CH * ccols], mybir.dt.int16,
                           kind="ExternalInput")
    meta_t = nc.dram_tensor("meta", [P, CAP_ROWS + 2 * ncol], mybir.dt.float32,
                            kind="ExternalInput")
    out_t = nc.dram_tensor("out", [G * CAP_ROWS, D], mybir.dt.float32,
                           kind="ExternalOutput")

    with tile.TileContext(nc) as tc:
        with (
            tc.tile_pool(name="const", bufs=1) as const_pool,
            tc.tile_pool(name="tokp", bufs=tok_bufs) as tok_pool,
            tc.tile_pool(name="work", bufs=work_bufs) as work_pool,
            tc.tile_pool(name="psum", bufs=psum_bufs, space="PSUM") as psum_pool,
        ):
            idx_sb = const_pool.tile([P, n_sg * CH * ccols], mybir.dt.int16)
            nc.sync.dma_start(idx_sb[:], idx_t[:])
            meta_sb = const_pool.tile([P, CAP_ROWS + 2 * ncol], mybir.dt.float32)
            nc.sync.dma_start(meta_sb[:], meta_t[:])
            iota_f = meta_sb[:, 0:CAP_ROWS]
            rel_all = meta_sb[:, CAP_ROWS : CAP_ROWS + ncol]
            val_all = meta_sb[:, CAP_ROWS + ncol : CAP_ROWS + 2 * ncol]

            stage = const_pool.tile([P, G * D], mybir.dt.float32)
            out_v = out_t[:].rearrange("(g w) d -> w g d", w=CAP_ROWS)

            for sg in range(n_sg):
                toks = []
                for cc in range(CH):
                    tok = tok_pool.tile([P, SG * TPC, D], mybir.dt.float32,
                                        tag=f"tok{cc}")
                    col0 = (sg * CH + cc) * ccols
                    nc.gpsimd.dma_gather(
                        tok[:],
                        x_t[cc * chunk_rows : (cc + 1) * chunk_rows, :],
                        idx_sb[:, col0 : col0 + ccols],
                        n_call_tok,
                        n_call_tok,
                        D,
                        single_packet=False,
                    )
                    toks.append(tok)
                for dg in range(SG):
                    g = sg * SG + dg
                    acc = psum_pool.tile([CAP_ROWS, D], mybir.dt.float32,
                                         tag="acc")
                    nmm = CH * TPC
                    i_mm = 0
                    for cc in range(CH):
                        for j in range(TPC):
                            k = g * CH * TPC + cc * TPC + j
                            S = work_pool.tile([P, CAP_ROWS], mybir.dt.float32,
                                               tag="S")
                            nc.vector.tensor_scalar(
                                out=S[:], in0=iota_f,
                                scalar1=rel_all[:, k : k + 1],
                                scalar2=None, op0=mybir.AluOpType.is_equal,
                            )
                            tok_s = work_pool.tile([P, D], mybir.dt.float32,
                                                   tag="tok_s")
                            nc.vector.tensor_scalar(
                                out=tok_s[:],
                                in0=toks[cc][:, dg * TPC + j, :],
                                scalar1=val_all[:, k : k + 1],
                                scalar2=None, op0=mybir.AluOpType.mult,
                            )
                            nc.tensor.matmul(
                                acc[:], S[:], tok_s[:],
                                start=(i_mm == 0), stop=(i_mm == nmm - 1),
                            )
                            i_mm += 1
                    nc.vector.tensor_copy(
                        stage[:CAP_ROWS, g * D : (g + 1) * D], acc[:]
                    )
                g0, g1 = sg * SG, sg * SG + SG
                nc.sync.dma_start(
                    out_v[:CAP_ROWS, g0:g1, :],
                    stage[:CAP_ROWS, g0 * D : g1 * D].rearrange(
                        "w (g d) -> w g d", d=D
                    ),
                )
    nc.compile()
    return nc


def _legalize_waits(nc):
    """This walrus build accepts only ONE embedded sync-wait per instruction;
    split extras onto same-engine NoOps placed just before (the sequencer
    executes them in order, so blocking semantics are identical)."""
    import concourse.mybir as mybir

    for f in nc.m.functions:
        for blk in f.blocks:
            newlist = []
            for ins in blk.instructions:
                si = ins.sync_info
                ow = list(si.on_wait) if si else []
                if len(ow) > 1:
                    for i, w in enumerate(ow[:-1]):
                        nop = mybir.InstNoOp(name=f"{ins.name}_ws{i}", ins=[],
                                             outs=[])
                        nop.engine = ins.engine
                        nop.sync_info = mybir.SyncInfo(on_wait=[w], on_update=[])
                        newlist.append(nop)
                    ins.sync_info = mybir.SyncInfo(
                        on_wait=[ow[-1]], on_update=list(si.on_update)
                    )
                newlist.append(ins)
            blk.instructions[:] = newlist


_LAST_RESULTS = None  # BassKernelResults of the most recent run (for test.py)


def prepare(adj_rows, adj_cols, adj_vals, x):
    """Host preprocessing + program build. Returns (nc, in_maps, row_ofs,
    n_nodes, G)."""
    rows = np.asarray(adj_rows).astype(np.int64)
    cols = np.asarray(adj_cols).astype(np.int64)
    vals = np.asarray(adj_vals).astype(np.float32)
    xf = np.ascontiguousarray(np.asarray(x), dtype=np.float32)
    n_nodes = xf.shape[0]
    chunk_rows = -(-n_nodes // CH)
    n_x_pad = chunk_rows * CH
    if n_x_pad != n_nodes:
        xf = np.concatenate(
            [xf, np.zeros((n_x_pad - n_nodes, D), np.float32)], 0
        )

    # contiguous row ranges per core
    bounds = [round(i * n_nodes / N_CORES) for i in range(N_CORES + 1)]
    G = 0
    for i in range(N_CORES):
        G = max(G, _count_groups(rows, cols, bounds[i], bounds[i + 1],
                                 chunk_rows))
    G = -(-G // SG) * SG

    in_maps = []
    row_ofs = []
    for i in range(N_CORES):
        idx_all, meta, row_of = _pack_core(
            rows, cols, vals, bounds[i], bounds[i + 1], G, chunk_rows
        )
        in_maps.append({"x": xf, "idx": idx_all, "meta": meta})
        row_ofs.append(row_of)

    nc = _build_program(n_x_pad, G, chunk_rows)
    _legalize_waits(nc)
    return nc, in_maps, row_ofs, n_nodes, G


def _unshard(results, row_ofs, n_nodes, G):
    out = np.zeros((n_nodes, D), np.float32)
    for i in range(N_CORES):
        staged = results[i]["out"].reshape(G, CAP_ROWS, D)
        row_of = row_ofs[i]
        mask = row_of >= 0
        out[row_of[mask]] = staged[mask]
    return out


def kernel(adj_rows, adj_cols, adj_vals, x):
    global _LAST_RESULTS
    from concourse.bass_utils import run_bass_kernel_spmd

    nc, in_maps, row_ofs, n_nodes, G = prepare(adj_rows, adj_cols, adj_vals, x)
    res = run_bass_kernel_spmd(nc, in_maps, core_ids=list(range(N_CORES)))
    _LAST_RESULTS = res
    return _unshard(res.results, row_ofs, n_nodes, G)


# revision 6
# speedup vs baseline: 1.2602x; 1.2602x over previous
"""GCN layer (SpMM) Trainium2 kernel: out = segment_sum(vals * x[cols], rows).

Self-contained: host-side sharding/preprocessing + a uniform Bass/Tile
program run SPMD on 8 NeuronCores via bass_utils.run_bass_kernel_spmd.

Strategy (graph/data-parallel SpMM per the row-partition scheme):
  - adj_rows is sorted; each core takes a contiguous 1/8 row range (its
    edges are then contiguous).
  - Neighbor features are fetched with the SWDGE `dma_gather` (256B rows
    from HBM by index). Its indices are int16, so x is split into CH=4
    chunks of 25K rows; each edge is routed to its column's chunk.
  - Per core, whole rows are packed into groups (<=64 distinct rows, and
    <=256 edges per chunk); each (group, chunk) segment is padded to
    exactly 256 tokens. rel[token] = dense rank of its row in the group.
  - Gather calls batch SG=8 groups per chunk (2048 indices/call).
  - Reduction per group on-chip: for each 128-token tile, DVE builds a
    one-hot S[p,w] = (iota[w] == rel[p]) and scales the gathered rows by
    val; PE accumulates S^T @ (tok*val) into a PSUM [64,64] accumulator;
    the result is staged in SBUF and written out in batches.
  - Host scatters staged group rows back to out[row] (rows are unique
    across groups/cores, so no collisions).
"""
import numpy as np

D = 64
P = 128
N_CORES = 8
CH = 4
TPC = 2
SG = 8
CAP_ROWS = 64
N_QUEUES = 4


# ---------------------------------------------------------------- host side
def _pack_core(rows, cols, vals, r_lo, r_hi, G, chunk_rows):
    seg_cap = TPC * P
    e_lo = np.searchsorted(rows, r_lo, "left")
    e_hi = np.searchsorted(rows, r_hi, "left")
    r = rows[e_lo:e_hi].astype(np.int64)
    c = cols[e_lo:e_hi].astype(np.int64)
    v = vals[e_lo:e_hi].astype(np.float32)
    ch = c // chunk_rows
    n_rows_core = r_hi - r_lo
    rr = r - r_lo
    cum = np.zeros((CH, n_rows_core + 1), np.int64)
    for cc in range(CH):
        cum[cc, 1:] = np.cumsum(np.bincount(rr[ch == cc], minlength=n_rows_core))
    starts = []
    s = 0
    while s < n_rows_core:
        hi = min(s + CAP_ROWS, n_rows_core)
        k = hi - s
        for cc in range(CH):
            kk = np.searchsorted(cum[cc, s + 1 : hi + 1] - cum[cc, s],
                                 seg_cap, "right")
            k = min(k, kk)
        if k == 0:
            raise ValueError("row degree exceeds segment capacity")
        starts.append((s, k))
        s += k
    assert len(starts) <= G, (len(starts), G)

    order = np.argsort(ch, kind="stable")
    r_s, c_s, v_s, ch_s = rr[order], c[order], v[order], ch[order]
    chunk_lo = np.searchsorted(ch_s, np.arange(CH), "left")
    chunk_hi = np.searchsorted(ch_s, np.arange(CH), "right")

    idx_lin = np.zeros((G, CH, seg_cap), np.int16)
    rel_lin = np.zeros((G, CH, seg_cap), np.float32)
    val_lin = np.zeros((G, CH, seg_cap), np.float32)
    row_of = np.full((G, CAP_ROWS), -1, np.int64)

    rank = np.zeros(n_rows_core, np.int64)
    gid = np.zeros(n_rows_core, np.int64)
    for g, (s, k) in enumerate(starts):
        rank[s : s + k] = np.arange(k)
        gid[s : s + k] = g
        row_of[g, :k] = r_lo + s + np.arange(k)

    n_groups_real = len(starts)
    for cc in range(CH):
        lo, hi = chunk_lo[cc], chunk_hi[cc]
        rcc = r_s[lo:hi]
        icc = (c_s[lo:hi] - cc * chunk_rows).astype(np.int16)
        vcc = v_s[lo:hi]
        relcc = rank[rcc].astype(np.float32)
        gcc = gid[rcc]
        grp_start = np.searchsorted(gcc, np.arange(n_groups_real), "left")
        pos = np.arange(hi - lo) - grp_start[gcc]
        idx_lin[gcc, cc, pos] = icc
        rel_lin[gcc, cc, pos] = relcc
        val_lin[gcc, cc, pos] = vcc

    ncol = G * CH * TPC
    rel_all = np.zeros((P, ncol), np.float32)
    val_all = np.zeros((P, ncol), np.float32)
    k_col = (
        np.arange(G)[:, None, None] * (CH * TPC)
        + np.arange(CH)[None, :, None] * TPC
        + (np.arange(seg_cap)[None, None, :] // P)
    )
    p_col = np.arange(seg_cap)[None, None, :] % P
    rel_all[p_col, k_col] = rel_lin
    val_all[p_col, k_col] = val_lin

    n_call_tok = SG * seg_cap
    n_sg = G // SG
    ccols = n_call_tok // 16
    idx_all = np.zeros((P, n_sg * CH * ccols), np.int16)
    for sg in range(n_sg):
        for cc in range(CH):
            lin = idx_lin[sg * SG : (sg + 1) * SG, cc, :].reshape(-1)
            blk = lin.reshape(ccols, 16).T
            col0 = (sg * CH + cc) * ccols
            idx_all[:, col0 : col0 + ccols] = np.tile(blk, (P // 16, 1))

    iota = np.broadcast_to(np.arange(CAP_ROWS, dtype=np.float32), (P, CAP_ROWS))
    meta = np.ascontiguousarray(np.concatenate([iota, rel_all, val_all], 1))
    return idx_all, meta, row_of


def _count_groups(rows, cols, r_lo, r_hi, chunk_rows):
    seg_cap = TPC * P
    e_lo = np.searchsorted(rows, r_lo, "left")
    e_hi = np.searchsorted(rows, r_hi, "left")
    r = rows[e_lo:e_hi].astype(np.int64) - r_lo
    c = cols[e_lo:e_hi].astype(np.int64)
    ch = c // chunk_rows
    n_rows_core = r_hi - r_lo
    cum = np.zeros((CH, n_rows_core + 1), np.int64)
    for cc in range(CH):
        cum[cc, 1:] = np.cumsum(np.bincount(r[ch == cc], minlength=n_rows_core))
    s, n = 0, 0
    while s < n_rows_core:
        hi = min(s + CAP_ROWS, n_rows_core)
        k = hi - s
        for cc in range(CH):
            kk = np.searchsorted(cum[cc, s + 1 : hi + 1] - cum[cc, s],
                                 seg_cap, "right")
            k = min(k, kk)
        if k == 0:
            raise ValueError("row degree exceeds segment capacity")
        s += k
        n += 1
    return n


# ---------------------------------------------------------------- device side
def _build_program(n_x_rows_padded, G, chunk_rows, tok_bufs=2, work_bufs=4,
                   psum_bufs=4):
    import concourse.bacc as bacc
    import concourse.mybir as mybir
    import concourse.tile as tile

    n_sg = G // SG
    seg_cap = TPC * P
    n_call_tok = SG * seg_cap
    ccols = n_call_tok // 16
    ncol = G * CH * TPC

    nc = bacc.Bacc(None, num_swdge_queues=N_QUEUES)
    x_t = nc.dram_tensor("x", [n_x_rows_padded, D], mybir.dt.float32,
                         kind="ExternalInput")
    idx_t = nc.dram_tensor("idx", [P, n_sg * CH * ccols], mybir.dt.int16,
                           kind="ExternalInput")
    meta_t = nc.dram_tensor("meta", [P, CAP_ROWS + 2 * ncol], mybir.dt.float32,
                            kind="ExternalInput")
    out_t = nc.dram_tensor("out", [G * CAP_ROWS, D], mybir.dt.float32,
                           kind="ExternalOutput")

    with tile.TileContext(nc) as tc:
        with (
            tc.tile_pool(name="const", bufs=1) as const_pool,
            tc.tile_pool(name="tokp", bufs=tok_bufs) as tok_pool,
            tc.tile_pool(name="work", bufs=work_bufs) as work_pool,
            tc.tile_pool(name="psum", bufs=psum_bufs, space="PSUM") as psum_pool,
        ):
            idx_sb = const_pool.tile([P, n_sg * CH * ccols], mybir.dt.int16)
            nc.sync.dma_start(idx_sb[:], idx_t[:])
            meta_sb = const_pool.tile([P, CAP_ROWS + 2 * ncol], mybir.dt.float32)
            nc.sync.dma_start(meta_sb[:], meta_t[:])
            iota_f = meta_sb[:, 0:CAP_ROWS]
            rel_all = meta_sb[:, CAP_ROWS : CAP_ROWS + ncol]
            val_all = meta_sb[:, CAP_ROWS + ncol : CAP_ROWS + 2 * ncol]

            stage = const_pool.tile([P, G * D], mybir.dt.float32)
            out_v = out_t[:].rearrange("(g w) d -> w g d", w=CAP_ROWS)

            for sg in range(n_sg):
                toks = []
                for cc in range(CH):
                    tok = tok_pool.tile([P, SG * TPC, D], mybir.dt.float32,
                                        tag=f"tok{cc}")
                    col0 = (sg * CH + cc) * ccols
                    nc.gpsimd.dma_gather(
                        tok[:],
                        x_t[cc * chunk_rows : (cc + 1) * chunk_rows, :],
                        idx_sb[:, col0 : col0 + ccols],
                        n_call_tok,
                        n_call_tok,
                        D,
                        single_packet=False,
                        queue_num=cc % N_QUEUES,
                    )
                    toks.append(tok)
                for dg in range(SG):
                    g = sg * SG + dg
                    acc = psum_pool.tile([CAP_ROWS, D], mybir.dt.float32,
                                         tag="acc")
                    nmm = CH * TPC
                    i_mm = 0
                    for cc in range(CH):
                        for j in range(TPC):
                            k = g * CH * TPC + cc * TPC + j
                            S = work_pool.tile([P, CAP_ROWS], mybir.dt.float32,
                                               tag="S")
                            nc.vector.tensor_scalar(
                                out=S[:], in0=iota_f,
                                scalar1=rel_all[:, k : k + 1],
                                scalar2=None, op0=mybir.AluOpType.is_equal,
                            )
                            tok_s = work_pool.tile([P, D], mybir.dt.float32,
                                                   tag="tok_s")
                            nc.vector.tensor_scalar(
                                out=tok_s[:],
                                in0=toks[cc][:, dg * TPC + j, :],
                                scalar1=val_all[:, k : k + 1],
                                scalar2=None, op0=mybir.AluOpType.mult,
                            )
                            nc.tensor.matmul(
                                acc[:], S[:], tok_s[:],
                                start=(i_mm == 0), stop=(i_mm == nmm - 1),
                            )
                            i_mm += 1
                    nc.vector.tensor_copy(
                        stage[:CAP_ROWS, g * D : (g + 1) * D], acc[:]
                    )
                g0, g1 = sg * SG, sg * SG + SG
                nc.sync.dma_start(
                    out_v[:CAP_ROWS, g0:g1, :],
                    stage[:CAP_ROWS, g0 * D : g1 * D].rearrange(
                        "w (g d) -> w g d", d=D
                    ),
                )
    nc.compile()
    return nc


def _legalize_waits(nc):
    """This walrus build accepts only ONE embedded sync-wait per instruction;
    split extras onto same-engine NoOps placed just before (the sequencer
    executes them in order, so blocking semantics are identical)."""
    import concourse.mybir as mybir

    for f in nc.m.functions:
        for blk in f.blocks:
            newlist = []
            for ins in blk.instructions:
                si = ins.sync_info
                ow = list(si.on_wait) if si else []
                if len(ow) > 1:
                    for i, w in enumerate(ow[:-1]):
                        nop = mybir.InstNoOp(name=f"{ins.name}_ws{i}", ins=[],
                                             outs=[])
                        nop.engine = ins.engine
                        nop.sync_info = mybir.SyncInfo(on_wait=[w], on_update=[])
                        newlist.append(nop)
                    ins.sync_info = mybir.SyncInfo(
                        on_wait=[ow[-1]], on_update=list(si.on_update)
                    )
                newlist.append(ins)
            blk.instructions[:] = newlist


_LAST_RESULTS = None  # BassKernelResults of the most recent run (for test.py)


def prepare(adj_rows, adj_cols, adj_vals, x):
    """Host preprocessing + program build. Returns (nc, in_maps, row_ofs,
    n_nodes, G)."""
    rows = np.asarray(adj_rows).astype(np.int64)
    cols = np.asarray(adj_cols).astype(np.int64)
    vals = np.asarray(adj_vals).astype(np.float32)
    xf = np.ascontiguousarray(np.asarray(x), dtype=np.float32)
    n_nodes = xf.shape[0]
    chunk_rows = -(-n_nodes // CH)
    n_x_pad = chunk_rows * CH
    if n_x_pad != n_nodes:
        xf = np.concatenate(
            [xf, np.zeros((n_x_pad - n_nodes, D), np.float32)], 0
        )

    # contiguous row ranges per core
    bounds = [round(i * n_nodes / N_CORES) for i in range(N_CORES + 1)]
    G = 0
    for i in range(N_CORES):
        G = max(G, _count_groups(rows, cols, bounds[i], bounds[i + 1],
                                 chunk_rows))
    G = -(-G // SG) * SG

    in_maps = []
    row_ofs = []
    for i in range(N_CORES):
        idx_all, meta, row_of = _pack_core(
            rows, cols, vals, bounds[i], bounds[i + 1], G, chunk_rows
        )
        in_maps.append({"x": xf, "idx": idx_all, "meta": meta})
        row_ofs.append(row_of)

    nc = _build_program(n_x_pad, G, chunk_rows)
    _legalize_waits(nc)
    return nc, in_maps, row_ofs, n_nodes, G


def _unshard(results, row_ofs, n_nodes, G):
    out = np.zeros((n_nodes, D), np.float32)
    for i in range(N_CORES):
        staged = results[i]["out"].reshape(G, CAP_ROWS, D)
        row_of = row_ofs[i]
        mask = row_of >= 0
        out[row_of[mask]] = staged[mask]
    return out


def kernel(adj_rows, adj_cols, adj_vals, x):
    global _LAST_RESULTS
    from concourse.bass_utils import run_bass_kernel_spmd

    nc, in_maps, row_ofs, n_nodes, G = prepare(adj_rows, adj_cols, adj_vals, x)
    res = run_bass_kernel_spmd(nc, in_maps, core_ids=list(range(N_CORES)))
    _LAST_RESULTS = res
    return _unshard(res.results, row_ofs, n_nodes, G)


# revision 8
# speedup vs baseline: 4.0657x; 3.2263x over previous
"""GCN layer (SpMM) Trainium2 kernel: out = segment_sum(vals * x[cols], rows).

Self-contained: host-side sharding/preprocessing + a uniform Bass/Tile
program run SPMD on 8 NeuronCores via bass_utils.run_bass_kernel_spmd.

Strategy (graph/data-parallel SpMM per the row-partition scheme):
  - adj_rows is sorted; each core takes a contiguous 1/8 row range (its
    edges are then contiguous).
  - Neighbor features are fetched with the SWDGE `dma_gather` (256B rows
    from HBM by index). Its indices are int16, so x is split into CH=4
    chunks of 25K rows; each edge is routed to its column's chunk.
  - Per core, whole rows are packed into groups (<=64 distinct rows, and
    <=256 edges per chunk); each (group, chunk) segment is padded to
    exactly 256 tokens. rel[token] = dense rank of its row in the group.
  - Gather calls batch SG=8 groups per chunk (2048 indices/call).
  - Reduction per group on-chip: for each 128-token tile, DVE builds a
    one-hot S[p,w] = (iota[w] == rel[p]) and scales the gathered rows by
    val; PE accumulates S^T @ (tok*val) into a PSUM [64,64] accumulator;
    the result is staged in SBUF and written out in batches.
  - Host scatters staged group rows back to out[row] (rows are unique
    across groups/cores, so no collisions).
"""
import numpy as np

D = 64
P = 128
N_CORES = 8
CH = 4
TPC = 2
SG = 8
CAP_ROWS = 64
N_QUEUES = 4


# ---------------------------------------------------------------- host side
def _pack_core(rows, cols, vals, r_lo, r_hi, G, chunk_rows):
    seg_cap = TPC * P
    e_lo = np.searchsorted(rows, r_lo, "left")
    e_hi = np.searchsorted(rows, r_hi, "left")
    r = rows[e_lo:e_hi].astype(np.int64)
    c = cols[e_lo:e_hi].astype(np.int64)
    v = vals[e_lo:e_hi].astype(np.float32)
    ch = c // chunk_rows
    n_rows_core = r_hi - r_lo
    rr = r - r_lo
    cum = np.zeros((CH, n_rows_core + 1), np.int64)
    for cc in range(CH):
        cum[cc, 1:] = np.cumsum(np.bincount(rr[ch == cc], minlength=n_rows_core))
    starts = []
    s = 0
    while s < n_rows_core:
        hi = min(s + CAP_ROWS, n_rows_core)
        k = hi - s
        for cc in range(CH):
            kk = np.searchsorted(cum[cc, s + 1 : hi + 1] - cum[cc, s],
                                 seg_cap, "right")
            k = min(k, kk)
        if k == 0:
            raise ValueError("row degree exceeds segment capacity")
        starts.append((s, k))
        s += k
    assert len(starts) <= G, (len(starts), G)

    order = np.argsort(ch, kind="stable")
    r_s, c_s, v_s, ch_s = rr[order], c[order], v[order], ch[order]
    chunk_lo = np.searchsorted(ch_s, np.arange(CH), "left")
    chunk_hi = np.searchsorted(ch_s, np.arange(CH), "right")

    idx_lin = np.zeros((G, CH, seg_cap), np.int16)
    rel_lin = np.zeros((G, CH, seg_cap), np.float32)
    val_lin = np.zeros((G, CH, seg_cap), np.float32)
    row_of = np.full((G, CAP_ROWS), -1, np.int64)

    rank = np.zeros(n_rows_core, np.int64)
    gid = np.zeros(n_rows_core, np.int64)
    for g, (s, k) in enumerate(starts):
        rank[s : s + k] = np.arange(k)
        gid[s : s + k] = g
        row_of[g, :k] = r_lo + s + np.arange(k)

    n_groups_real = len(starts)
    for cc in range(CH):
        lo, hi = chunk_lo[cc], chunk_hi[cc]
        rcc = r_s[lo:hi]
        icc = (c_s[lo:hi] - cc * chunk_rows).astype(np.int16)
        vcc = v_s[lo:hi]
        relcc = rank[rcc].astype(np.float32)
        gcc = gid[rcc]
        grp_start = np.searchsorted(gcc, np.arange(n_groups_real), "left")
        pos = np.arange(hi - lo) - grp_start[gcc]
        idx_lin[gcc, cc, pos] = icc
        rel_lin[gcc, cc, pos] = relcc
        val_lin[gcc, cc, pos] = vcc

    ncol = G * CH * TPC
    # rel: group-major columns (g, cc, j) -> one batched S-build per group
    rel_all = np.zeros((P, ncol), np.float32)
    k_col = (
        np.arange(G)[:, None, None] * (CH * TPC)
        + np.arange(CH)[None, :, None] * TPC
        + (np.arange(seg_cap)[None, None, :] // P)
    )
    p_col = np.arange(seg_cap)[None, None, :] % P
    rel_all[p_col, k_col] = rel_lin
    # val: call-major columns ((sg, cc), dg, j) -> one batched scale per call,
    # matching the gather call's tile order
    val_all = np.zeros((P, ncol), np.float32)
    sg_id = np.arange(G)[:, None, None] // SG
    dg_id = np.arange(G)[:, None, None] % SG
    kv_col = (
        (sg_id * CH + np.arange(CH)[None, :, None]) * (SG * TPC)
        + dg_id * TPC
        + (np.arange(seg_cap)[None, None, :] // P)
    )
    val_all[p_col, kv_col] = val_lin

    n_call_tok = SG * seg_cap
    n_sg = G // SG
    ccols = n_call_tok // 16
    idx_all = np.zeros((P, n_sg * CH * ccols), np.int16)
    for sg in range(n_sg):
        for cc in range(CH):
            lin = idx_lin[sg * SG : (sg + 1) * SG, cc, :].reshape(-1)
            blk = lin.reshape(ccols, 16).T
            col0 = (sg * CH + cc) * ccols
            idx_all[:, col0 : col0 + ccols] = np.tile(blk, (P // 16, 1))

    iota = np.broadcast_to(np.arange(CAP_ROWS, dtype=np.float32), (P, CAP_ROWS))
    meta = np.ascontiguousarray(np.concatenate([iota, rel_all, val_all], 1))
    return idx_all, meta, row_of


def _count_groups(rows, cols, r_lo, r_hi, chunk_rows):
    seg_cap = TPC * P
    e_lo = np.searchsorted(rows, r_lo, "left")
    e_hi = np.searchsorted(rows, r_hi, "left")
    r = rows[e_lo:e_hi].astype(np.int64) - r_lo
    c = cols[e_lo:e_hi].astype(np.int64)
    ch = c // chunk_rows
    n_rows_core = r_hi - r_lo
    cum = np.zeros((CH, n_rows_core + 1), np.int64)
    for cc in range(CH):
        cum[cc, 1:] = np.cumsum(np.bincount(r[ch == cc], minlength=n_rows_core))
    s, n = 0, 0
    while s < n_rows_core:
        hi = min(s + CAP_ROWS, n_rows_core)
        k = hi - s
        for cc in range(CH):
            kk = np.searchsorted(cum[cc, s + 1 : hi + 1] - cum[cc, s],
                                 seg_cap, "right")
            k = min(k, kk)
        if k == 0:
            raise ValueError("row degree exceeds segment capacity")
        s += k
        n += 1
    return n


# ---------------------------------------------------------------- device side
def _build_program(n_x_rows_padded, G, chunk_rows, tok_bufs=2, work_bufs=4,
                   psum_bufs=4):
    import concourse.bacc as bacc
    import concourse.mybir as mybir
    import concourse.tile as tile

    n_sg = G // SG
    seg_cap = TPC * P
    n_call_tok = SG * seg_cap
    ccols = n_call_tok // 16
    ncol = G * CH * TPC

    nc = bacc.Bacc(None, num_swdge_queues=N_QUEUES)
    x_t = nc.dram_tensor("x", [n_x_rows_padded, D], mybir.dt.float32,
                         kind="ExternalInput")
    idx_t = nc.dram_tensor("idx", [P, n_sg * CH * ccols], mybir.dt.int16,
                           kind="ExternalInput")
    meta_t = nc.dram_tensor("meta", [P, CAP_ROWS + 2 * ncol], mybir.dt.float32,
                            kind="ExternalInput")
    out_t = nc.dram_tensor("out", [G * CAP_ROWS, D], mybir.dt.float32,
                           kind="ExternalOutput")

    with tile.TileContext(nc) as tc:
        with (
            tc.tile_pool(name="const", bufs=1) as const_pool,
            tc.tile_pool(name="tokp", bufs=tok_bufs) as tok_pool,
            tc.tile_pool(name="work", bufs=work_bufs) as work_pool,
            tc.tile_pool(name="psum", bufs=psum_bufs, space="PSUM") as psum_pool,
        ):
            idx_sb = const_pool.tile([P, n_sg * CH * ccols], mybir.dt.int16)
            nc.sync.dma_start(idx_sb[:], idx_t[:])
            meta_sb = const_pool.tile([P, CAP_ROWS + 2 * ncol], mybir.dt.float32)
            nc.sync.dma_start(meta_sb[:], meta_t[:])
            iota_f = meta_sb[:, 0:CAP_ROWS]
            rel_all = meta_sb[:, CAP_ROWS : CAP_ROWS + ncol]
            val_all = meta_sb[:, CAP_ROWS + ncol : CAP_ROWS + 2 * ncol]

            stage = const_pool.tile([P, G * D], mybir.dt.float32)
            out_v = out_t[:].rearrange("(g w) d -> w g d", w=CAP_ROWS)

            for sg in range(n_sg):
                toks = []
                for cc in range(CH):
                    tok = tok_pool.tile([P, SG * TPC, D], mybir.dt.float32,
                                        tag=f"tok{cc}")
                    col0 = (sg * CH + cc) * ccols
                    nc.gpsimd.dma_gather(
                        tok[:],
                        x_t[cc * chunk_rows : (cc + 1) * chunk_rows, :],
                        idx_sb[:, col0 : col0 + ccols],
                        n_call_tok,
                        n_call_tok,
                        D,
                        single_packet=False,
                        queue_num=cc % N_QUEUES,
                    )
                    # scale the whole call's tokens by val in one DVE op
                    tok_s = tok_pool.tile([P, SG * TPC, D], mybir.dt.float32,
                                          tag=f"toks{cc}")
                    kv0 = (sg * CH + cc) * (SG * TPC)
                    nc.vector.tensor_tensor(
                        out=tok_s[:],
                        in0=tok[:],
                        in1=val_all[:, kv0 : kv0 + SG * TPC]
                        .unsqueeze(2)
                        .broadcast_to([P, SG * TPC, D]),
                        op=mybir.AluOpType.mult,
                    )
                    toks.append(tok_s)
                for dg in range(SG):
                    g = sg * SG + dg
                    k0 = g * CH * TPC
                    # all CH*TPC one-hot S matrices for this group in one op
                    S = work_pool.tile([P, CH * TPC, CAP_ROWS],
                                       mybir.dt.float32, tag="S")
                    nc.vector.tensor_tensor(
                        out=S[:],
                        in0=iota_f.unsqueeze(1)
                        .broadcast_to([P, CH * TPC, CAP_ROWS]),
                        in1=rel_all[:, k0 : k0 + CH * TPC]
                        .unsqueeze(2)
                        .broadcast_to([P, CH * TPC, CAP_ROWS]),
                        op=mybir.AluOpType.is_equal,
                    )
                    acc = psum_pool.tile([CAP_ROWS, D], mybir.dt.float32,
                                         tag="acc")
                    nmm = CH * TPC
                    i_mm = 0
                    for cc in range(CH):
                        for j in range(TPC):
                            nc.tensor.matmul(
                                acc[:], S[:, cc * TPC + j, :],
                                toks[cc][:, dg * TPC + j, :],
                                start=(i_mm == 0), stop=(i_mm == nmm - 1),
                            )
                            i_mm += 1
                    nc.vector.tensor_copy(
                        stage[:CAP_ROWS, g * D : (g + 1) * D], acc[:]
                    )
                g0, g1 = sg * SG, sg * SG + SG
                nc.sync.dma_start(
                    out_v[:CAP_ROWS, g0:g1, :],
                    stage[:CAP_ROWS, g0 * D : g1 * D].rearrange(
                        "w (g d) -> w g d", d=D
                    ),
                )
    nc.compile()
    return nc


def _legalize_waits(nc):
    """This walrus build accepts only ONE embedded sync-wait per instruction;
    split extras onto same-engine NoOps placed just before (the sequencer
    executes them in order, so blocking semantics are identical)."""
    import concourse.mybir as mybir

    for f in nc.m.functions:
        for blk in f.blocks:
            newlist = []
            for ins in blk.instructions:
                si = ins.sync_info
                ow = list(si.on_wait) if si else []
                if len(ow) > 1:
                    for i, w in enumerate(ow[:-1]):
                        nop = mybir.InstNoOp(name=f"{ins.name}_ws{i}", ins=[],
                                             outs=[])
                        nop.engine = ins.engine
                        nop.sync_info = mybir.SyncInfo(on_wait=[w], on_update=[])
                        newlist.append(nop)
                    ins.sync_info = mybir.SyncInfo(
                        on_wait=[ow[-1]], on_update=list(si.on_update)
                    )
                newlist.append(ins)
            blk.instructions[:] = newlist


_LAST_RESULTS = None  # BassKernelResults of the most recent run (for test.py)


def prepare(adj_rows, adj_cols, adj_vals, x):
    """Host preprocessing + program build. Returns (nc, in_maps, row_ofs,
    n_nodes, G)."""
    rows = np.asarray(adj_rows).astype(np.int64)
    cols = np.asarray(adj_cols).astype(np.int64)
    vals = np.asarray(adj_vals).astype(np.float32)
    xf = np.ascontiguousarray(np.asarray(x), dtype=np.float32)
    n_nodes = xf.shape[0]
    chunk_rows = -(-n_nodes // CH)
    n_x_pad = chunk_rows * CH
    if n_x_pad != n_nodes:
        xf = np.concatenate(
            [xf, np.zeros((n_x_pad - n_nodes, D), np.float32)], 0
        )

    # contiguous row ranges per core
    bounds = [round(i * n_nodes / N_CORES) for i in range(N_CORES + 1)]
    G = 0
    for i in range(N_CORES):
        G = max(G, _count_groups(rows, cols, bounds[i], bounds[i + 1],
                                 chunk_rows))
    G = -(-G // SG) * SG

    in_maps = []
    row_ofs = []
    for i in range(N_CORES):
        idx_all, meta, row_of = _pack_core(
            rows, cols, vals, bounds[i], bounds[i + 1], G, chunk_rows
        )
        in_maps.append({"x": xf, "idx": idx_all, "meta": meta})
        row_ofs.append(row_of)

    nc = _build_program(n_x_pad, G, chunk_rows)
    _legalize_waits(nc)
    return nc, in_maps, row_ofs, n_nodes, G


def _unshard(results, row_ofs, n_nodes, G):
    out = np.zeros((n_nodes, D), np.float32)
    for i in range(N_CORES):
        staged = results[i]["out"].reshape(G, CAP_ROWS, D)
        row_of = row_ofs[i]
        mask = row_of >= 0
        out[row_of[mask]] = staged[mask]
    return out


def kernel(adj_rows, adj_cols, adj_vals, x):
    global _LAST_RESULTS
    from concourse.bass_utils import run_bass_kernel_spmd

    nc, in_maps, row_ofs, n_nodes, G = prepare(adj_rows, adj_cols, adj_vals, x)
    res = run_bass_kernel_spmd(nc, in_maps, core_ids=list(range(N_CORES)))
    _LAST_RESULTS = res
    return _unshard(res.results, row_ofs, n_nodes, G)
